# revision 27
# baseline (speedup 1.0000x reference)
"""Trainium2 Bass kernel for 12-head attention (SEQ=4096, D=768), 8-core SPMD.

Sharding: head-parallel with a sequence-split remainder. Core c owns full head
A_c = c and the half of head B_c = 8 + c//2 selected by (c % 2). Upper-half
cores receive a sequence-rolled copy of x so every core's program computes
local queries [0, 2048) for its B head (pure SPMD, no divergent control flow).
Each core returns a partial output projection [768, 4096]; the host un-rolls,
transposes and sums the 8 partials.

Per-core dataflow (matmuls in fp16, softmax internals in fp32):
  x^T -> QKV^T projections -> scores S^T[j,i] = K^T(lhsT) x Q^T(rhs), with the
  two W-chunks of a pair computed concurrently on disjoint PE row groups
  -> exp on ScalarE (scale=1/8 folded in; no max subtraction, scores within
  [-10, 10]) -> attn@V with [V | ones] as the stationary operand so softmax
  denominators fall out as an extra PSUM row -> normalize via a K=2 broadcast
  matmul + fast reciprocal -> output projection.
"""

import numpy as np

N_CORES = 8
N_HEADS = 12
HEAD_DIM = 64
N_FEATS = 768
SEQ = 4096
FCH = N_FEATS // 128  # contraction chunks of the feature dim
W = 1024              # i-chunk width (exp granularity)
NIC = SEQ // W
NJB = SEQ // 128      # key blocks
NH = W // 512         # 512-wide matmul sub-chunks per i-chunk

_PROGRAM = None
LAST_RESULT = None


def _build_program():
    import concourse.tile as tile
    from concourse import bacc, mybir

    f32 = mybir.dt.float32
    f32r = mybir.dt.float32r
    f16 = mybir.dt.float16
    EXP = mybir.ActivationFunctionType.Exp

    nc = bacc.Bacc("TRN2", target_bir_lowering=False, debug=False,
                   num_devices=N_CORES)

    xt_d = nc.dram_tensor("xt", [N_FEATS, SEQ], f16, kind="ExternalInput").ap()
    wqk_d = nc.dram_tensor("wqk", [N_FEATS, 256], f16, kind="ExternalInput").ap()
    wv_d = nc.dram_tensor("wv", [N_FEATS, 128], f16, kind="ExternalInput").ap()
    wo_d = nc.dram_tensor("wo", [128, N_FEATS], f16, kind="ExternalInput").ap()
    sel_d = nc.dram_tensor("sel", [2, 128], f32r, kind="ExternalInput").ap()
    vconst_d = nc.dram_tensor("vconst", [128, NJB, 3], f16, kind="ExternalInput").ap()
    ones2_d = nc.dram_tensor("ones2", [2, SEQ // 2], f32r, kind="ExternalInput").ap()
    id_d = nc.dram_tensor("ident", [128, 128], f16, kind="ExternalInput").ap()
    out_d = nc.dram_tensor("out", [N_FEATS, SEQ], f32, kind="ExternalOutput").ap()

    with tile.TileContext(nc) as tc:
        with tc.tile_pool(name="persist", bufs=1) as pp:
            wqk_sb = pp.tile([128, FCH, 256], f16)
            wv_sb = pp.tile([128, FCH, 128], f16)
            wo_sb = pp.tile([128, N_FEATS], f16)
            sel_sb = pp.tile([66, 128], f32r)
            id_sb = pp.tile([128, 128], f16)
            nc.sync.dma_start(out=wqk_sb[:], in_=wqk_d.rearrange("(c p) m -> p c m", p=128))
            nc.gpsimd.dma_start(out=wv_sb[:], in_=wv_d.rearrange("(c p) m -> p c m", p=128))
            nc.gpsimd.dma_start(out=wo_sb[:], in_=wo_d[:])
            nc.gpsimd.dma_start(out=sel_sb[64:66, :], in_=sel_d[:])
            nc.gpsimd.dma_start(out=id_sb[:], in_=id_d[:])

            NCH = SEQ // 512
            # per-chunk tiles give the scheduler precise dependencies so the
            # attention stream starts as soon as its first chunks are ready
            q_t = [pp.tile([128, 512], f16, name=f"q_t{i}") for i in range(NCH)]
            k_t = [pp.tile([128, 512], f16, name=f"k_t{i}") for i in range(NCH)]
            q2_t = [pp.tile([128, 512], f16, name=f"q2_t{i}") for i in range(NCH)]
            k2_t = [pp.tile([128, 512], f16, name=f"k2_t{i}") for i in range(NCH)]
            vA_t = [pp.tile([128, 65], f16, name=f"vA_t{j}") for j in range(NJB)]
            vB_t = [pp.tile([128, 66], f16, name=f"vB_t{j}") for j in range(NJB)]
            attn_out = pp.tile([128, SEQ], f32)  # rows 0-63 A dims, 64-127 B dims
            den = pp.tile([66, SEQ], f32r)       # rows 64 (A), 65 (B)

            for j in range(NJB):
                nc.gpsimd.dma_start(out=vA_t[j][:, 64:65], in_=vconst_d[:, j, 0:1])
                nc.gpsimd.dma_start(out=vB_t[j][:, 64:66], in_=vconst_d[:, j, 1:3])
            nc.gpsimd.dma_start(out=den[64:66, SEQ // 2:], in_=ones2_d[:])
            nc.vector.memset(attn_out[64:128, SEQ // 2:], 0.0)

            # ---- phases 1+2 fused: projections interleaved into the
            # attention stream (shared scores PSUM tag, precise chunk deps) ----
            ps_s_cm = tc.tile_pool(name="ps_s", space="PSUM", bufs=2)
            ps_s = ps_s_cm.__enter__()
            p1_cm = tc.tile_pool(name="ph1", bufs=1)
            p1 = p1_cm.__enter__()
            pe_cm = tc.tile_pool(name="exps", bufs=6)
            pe = pe_cm.__enter__()
            ps_ov_cm = tc.tile_pool(name="ps_ov", bufs=1, space="PSUM")
            ps_ov = ps_ov_cm.__enter__()

            xt = p1.tile([128, FCH, SEQ], f16)
            xt_r = xt_d.rearrange("(c p) n -> p c n", p=128)
            for k in range(FCH):
                nc.sync.dma_start(out=xt[:, k, :], in_=xt_r[:, k, :])
            vT_t = [p1.tile([128, 512], f16, name=f"vT_t{i}") for i in range(SEQ // 512)]

            def proj_qk(nch):
                pq = ps_s.tile([128, W], f32, tag="s", name=f"pjqk{nch}")
                for k in range(FCH):
                    nc.tensor.matmul(pq[:, 0:512], wqk_sb[:, k, 0:128],
                                     xt[:, k, nch * 512:(nch + 1) * 512],
                                     start=(k == 0), stop=(k == FCH - 1))
                    nc.tensor.matmul(pq[:, 512:1024], wqk_sb[:, k, 128:256],
                                     xt[:, k, nch * 512:(nch + 1) * 512],
                                     start=(k == 0), stop=(k == FCH - 1))
                nc.vector.tensor_copy(q_t[nch][:], pq[:, 0:512])
                nc.vector.tensor_copy(k_t[nch][:], pq[:, 512:1024])
                nc.gpsimd.dma_start(out=q2_t[nch][64:128, :], in_=q_t[nch][0:64, :])
                nc.gpsimd.dma_start(out=k2_t[nch][64:128, :], in_=k_t[nch][0:64, :])

            def proj_v(nch):
                pv = ps_s.tile([128, W], f32, tag="s", name=f"pjv{nch}")
                for k in range(FCH):
                    nc.tensor.matmul(pv[:, 0:512], wv_sb[:, k, :],
                                     xt[:, k, nch * 512:(nch + 1) * 512],
                                     start=(k == 0), stop=(k == FCH - 1))
                nc.vector.tensor_copy(vT_t[nch][:], pv[:, 0:512])
                ptt = ps_s.tile([128, 8, 128], f16, tag="s", name=f"ptr{nch}")
                for q in range(4):
                    jb = nch * 4 + q
                    nc.tensor.transpose(ptt[:, q, :], vT_t[nch][:, q * 128:(q + 1) * 128], id_sb[:])
                    nc.vector.tensor_copy(vA_t[jb][:, 0:64], ptt[:, q, 0:64])
                    nc.vector.tensor_copy(vB_t[jb][:, 0:64], ptt[:, q, 64:128])

            pairs = [
                (("A", 0, q_t, k_t, vA_t), ("B", 0, q_t, k_t, vB_t)),
                (("A", 1, q_t, k_t, vA_t), ("B", 1, q_t, k_t, vB_t)),
                (("A", 2, q_t, k_t, vA_t), ("A2", 3, q2_t, k2_t, vA_t)),
            ]

            # all projections up front (they share the scores PSUM tag, so no
            # bank-reuse barrier separates them from the attention stream)
            for n in range(8):
                proj_qk(n)
            for n in range(8):
                proj_v(n)

            for pi, (c1, c2) in enumerate(pairs):
                ov1 = ps_ov.tile([65, W], f32, tag="ov1", name=f"ov1_{pi}")
                ov2 = ps_ov.tile([66, W], f32, tag="ov2", name=f"ov2_{pi}")
                ovs = (ov1, ov2)
                for jb in range(NJB):
                    jc, jo = jb // 4, (jb % 4) * 128
                    etiles = []
                    for h in range(NH):
                        sp = ps_s.tile([128, W], f32, tag="s", name=f"s{pi}_{jb}_{h}")
                        for ci, (_, ic, qt, kt, _) in enumerate((c1, c2)):
                            base = ci * 64
                            nc.tensor.matmul(
                                sp[:, ci * 512:(ci + 1) * 512],
                                kt[jc][base:base + 64, jo:jo + 128],
                                qt[ic * NH + h][base:base + 64, :],
                                start=True, stop=True)
                        e = pe.tile([128, W], f16, tag="e", name=f"e{pi}_{jb}_{h}")
                        nc.scalar.activation(out=e[:], in_=sp[:], func=EXP, scale=0.125)
                        etiles.append(e)
                    for h, e in enumerate(etiles):
                        for ci, (_, ic, _, _, vt) in enumerate((c1, c2)):
                            m = vt[jb].shape[1]
                            nc.tensor.matmul(ovs[ci][:m, h * 512:(h + 1) * 512],
                                             vt[jb][:], e[:, ci * 512:(ci + 1) * 512],
                                             start=(jb == 0), stop=(jb == NJB - 1))
                # drain accumulators to SBUF
                (n1, ic1, _, _, _), (n2, ic2, _, _, _) = c1, c2
                p10, p20 = ic1 * W, ic2 * W
                nc.vector.tensor_copy(attn_out[0:64, p10:p10 + W], ov1[0:64, :])
                if n2 == "B":
                    nc.vector.tensor_copy(den[64:66, p20:p20 + W], ov2[64:66, :])
                    ovb_sb = pe.tile([64, W], f32, tag="ovb_sb", name=f"ovb_sb{pi}", bufs=2)
                    nc.vector.tensor_copy(ovb_sb[:], ov2[0:64, :])
                    nc.gpsimd.dma_start(out=attn_out[64:128, p20:p20 + W], in_=ovb_sb[:])
                    nc.vector.tensor_copy(den[64:65, p10:p10 + W], ov1[64:65, :])
                else:
                    nc.vector.tensor_copy(attn_out[0:64, p20:p20 + W], ov2[0:64, :].bitcast(f32))
                    nc.vector.tensor_copy(den[64:65, p10:p10 + W], ov1[64:65, :])
                    nc.vector.tensor_copy(den[64:65, p20:p20 + W], ov2[64:65, :])

            ps_ov_cm.__exit__(None, None, None)
            pe_cm.__exit__(None, None, None)
            p1_cm.__exit__(None, None, None)
            ps_s_cm.__exit__(None, None, None)

            # ---- phase 3: normalize + output projection ----
            with tc.tile_pool(name="ph3", bufs=2) as p3, \
                 tc.tile_pool(name="ps_bc", bufs=4, space="PSUM") as ps_bc, \
                 tc.tile_pool(name="ps_o", bufs=4, space="PSUM") as ps_o:
                NT = SEQ // 512
                bcs, rcs = [], []
                for t in range(NT):
                    t0 = t * 512
                    bc = ps_bc.tile([128, 512], f32, tag="bc", name=f"bc{t}", bufs=4)
                    nc.tensor.matmul(bc[:], sel_sb[64:66, :], den[64:66, t0:t0 + 512],
                                     start=True, stop=True)
                    rc = p3.tile([128, 512], f32, tag="rc", name=f"rc{t}", bufs=4)
                    nc.vector.reciprocal_approx_fast(out=rc[:], in_=bc[:])
                    rcs.append(rc)
                for t in range(NT):
                    t0 = t * 512
                    nm = p3.tile([128, 512], f16, tag="nm", name=f"nm{t}", bufs=3)
                    nc.vector.tensor_tensor(out=nm[:], in0=attn_out[:, t0:t0 + 512],
                                            in1=rcs[t][:], op=mybir.AluOpType.mult)
                    for fb in range(FCH):
                        po = ps_o.tile([128, 512], f32, tag="po", name=f"po{t}_{fb}")
                        nc.tensor.matmul(po[:], wo_sb[:, fb * 128:(fb + 1) * 128], nm[:],
                                         start=True, stop=True)
                        ob = p3.tile([128, 512], f32, tag="ob", name=f"ob{t}_{fb}", bufs=6)
                        if fb % 2 == 0:
                            nc.vector.tensor_copy(ob[:], po[:])
                        else:
                            nc.scalar.copy(ob[:], po[:])
                        out_q = nc.sync if fb % 2 == 0 else nc.gpsimd
                        out_q.dma_start(out=out_d[fb * 128:(fb + 1) * 128, t0:t0 + 512],
                                        in_=ob[:])

    nc.compile()
    return nc


def _get_program():
    global _PROGRAM
    if _PROGRAM is None:
        _PROGRAM = _build_program()
    return _PROGRAM


def kernel(x: np.ndarray, w_qkv: np.ndarray, w_out: np.ndarray) -> np.ndarray:
    global LAST_RESULT
    import os
    try:
        import antenv.axon_hooks  # noqa: F401
    except ImportError:
        # without the NTFF hook, a leaked BASS_TRACE=1 would crash the
        # axon trace path inside run_bass_kernel_spmd
        os.environ["BASS_NEVER_TRACE"] = "1"
    from concourse.bass_utils import run_bass_kernel_spmd

    nc = _get_program()
    x2 = np.ascontiguousarray(x[0], dtype=np.float32)          # [SEQ, F]
    w_qkv = np.asarray(w_qkv, dtype=np.float32)                # [2304, F]
    w_out = np.asarray(w_out, dtype=np.float32)                # [F, 768]

    # per-head slices of w_qkv rows: o = h*192 + d*3 + {0:q, 1:k, 2:v}
    def wslice(h, which):
        return w_qkv[h * 192 + which:(h + 1) * 192:3, :]       # [64, F]

    sel = np.zeros((2, 128), dtype=np.float32)
    sel[0, 0:64] = 1.0
    sel[1, 64:128] = 1.0
    ident = np.eye(128, dtype=np.float16)
    vconst = np.zeros((128, NJB, 3), dtype=np.float16)
    vconst[:, :, 0] = 1.0
    vconst[:, :, 2] = 1.0
    ones2 = np.ones((2, SEQ // 2), dtype=np.float32)

    xt_plain = np.ascontiguousarray(x2.T.astype(np.float16))   # [F, SEQ]
    xt_rolled = np.ascontiguousarray(np.roll(x2, -SEQ // 2, axis=0).T.astype(np.float16))

    in_maps = []
    rolls = []
    for c in range(N_CORES):
        hA = c
        hB = 8 + c // 2
        roll = (SEQ // 2) if (c % 2) else 0
        rolls.append(roll)
        wqk = np.ascontiguousarray(np.concatenate(
            [wslice(hA, 0), wslice(hB, 0), wslice(hA, 1), wslice(hB, 1)],
            axis=0).T.astype(np.float16))
        wv = np.ascontiguousarray(np.concatenate(
            [wslice(hA, 2), wslice(hB, 2)], axis=0).T.astype(np.float16))
        cols = list(range(hA * 64, hA * 64 + 64)) + list(range(hB * 64, hB * 64 + 64))
        wo = np.ascontiguousarray(w_out[:, cols].T.astype(np.float16))  # [128, F]
        in_maps.append({
            "xt": xt_rolled if roll else xt_plain,
            "wqk": wqk, "wv": wv, "wo": wo, "sel": sel, "ident": ident,
            "vconst": vconst, "ones2": ones2,
        })

    res = run_bass_kernel_spmd(nc, in_maps, list(range(N_CORES)))
    LAST_RESULT = res

    acc = np.zeros((SEQ, N_FEATS), dtype=np.float64)
    for c in range(N_CORES):
        part = res.results[c]["out"]                           # [F, SEQ]
        if rolls[c]:
            part = np.roll(part, rolls[c], axis=1)
        acc += part.T.astype(np.float64)
    return acc.astype(np.float32)[None]


# revision 28
# speedup vs baseline: 1.0464x; 1.0464x over previous
"""Trainium2 Bass kernel for 12-head attention (SEQ=4096, D=768), 8-core SPMD.

Sharding: head-parallel with a sequence-split remainder. Core c owns full head
A_c = c and the half of head B_c = 8 + c//2 selected by (c % 2). Upper-half
cores receive a sequence-rolled copy of x so every core's program computes
local queries [0, 2048) for its B head (pure SPMD, no divergent control flow).
Each core returns a partial output projection [768, 4096]; the host un-rolls,
transposes and sums the 8 partials.

Per-core dataflow (matmuls in fp16, softmax internals in fp32):
  x^T -> QKV^T projections -> scores S^T[j,i] = K^T(lhsT) x Q^T(rhs), with the
  two W-chunks of a pair computed concurrently on disjoint PE row groups
  -> exp on ScalarE (scale=1/8 folded in; no max subtraction, scores within
  [-10, 10]) -> attn@V with [V | ones] as the stationary operand so softmax
  denominators fall out as an extra PSUM row -> normalize via a K=2 broadcast
  matmul + fast reciprocal -> output projection.
"""

import numpy as np

N_CORES = 8
N_HEADS = 12
HEAD_DIM = 64
N_FEATS = 768
SEQ = 4096
FCH = N_FEATS // 128  # contraction chunks of the feature dim
W = 1024              # i-chunk width (exp granularity)
NIC = SEQ // W
NJB = SEQ // 128      # key blocks
NH = W // 512         # 512-wide matmul sub-chunks per i-chunk

_PROGRAM = None
LAST_RESULT = None


def _build_program():
    import concourse.tile as tile
    from concourse import bacc, mybir

    f32 = mybir.dt.float32
    f32r = mybir.dt.float32r
    f16 = mybir.dt.float16
    EXP = mybir.ActivationFunctionType.Exp

    nc = bacc.Bacc("TRN2", target_bir_lowering=False, debug=False,
                   num_devices=N_CORES)

    xt_d = nc.dram_tensor("xt", [N_FEATS, SEQ], f16, kind="ExternalInput").ap()
    wqk_d = nc.dram_tensor("wqk", [N_FEATS, 256], f16, kind="ExternalInput").ap()
    wv_d = nc.dram_tensor("wv", [N_FEATS, 128], f16, kind="ExternalInput").ap()
    wo_d = nc.dram_tensor("wo", [128, N_FEATS], f16, kind="ExternalInput").ap()
    sel_d = nc.dram_tensor("sel", [2, 128], f32r, kind="ExternalInput").ap()
    vconst_d = nc.dram_tensor("vconst", [128, NJB, 3], f16, kind="ExternalInput").ap()
    ones2_d = nc.dram_tensor("ones2", [2, SEQ // 2], f32r, kind="ExternalInput").ap()
    id_d = nc.dram_tensor("ident", [128, 128], f16, kind="ExternalInput").ap()
    out_d = nc.dram_tensor("out", [N_FEATS, SEQ], f32, kind="ExternalOutput").ap()

    with tile.TileContext(nc) as tc:
        with tc.tile_pool(name="persist", bufs=1) as pp:
            wqk_sb = pp.tile([128, FCH, 256], f16)
            wv_sb = pp.tile([128, FCH, 128], f16)
            wo_sb = pp.tile([128, N_FEATS], f16)
            sel_sb = pp.tile([66, 128], f32r)
            id_sb = pp.tile([128, 128], f16)
            nc.sync.dma_start(out=wqk_sb[:], in_=wqk_d.rearrange("(c p) m -> p c m", p=128))
            nc.gpsimd.dma_start(out=wv_sb[:], in_=wv_d.rearrange("(c p) m -> p c m", p=128))
            nc.gpsimd.dma_start(out=wo_sb[:], in_=wo_d[:])
            nc.gpsimd.dma_start(out=sel_sb[64:66, :], in_=sel_d[:])
            nc.gpsimd.dma_start(out=id_sb[:], in_=id_d[:])

            NCH = SEQ // 512
            # per-chunk tiles give the scheduler precise dependencies so the
            # attention stream starts as soon as its first chunks are ready
            q_t = [pp.tile([128, 512], f16, name=f"q_t{i}") for i in range(NCH)]
            k_t = [pp.tile([128, 512], f16, name=f"k_t{i}") for i in range(NCH)]
            q2_t = [pp.tile([128, 512], f16, name=f"q2_t{i}") for i in range(NCH)]
            k2_t = [pp.tile([128, 512], f16, name=f"k2_t{i}") for i in range(NCH)]
            vA_t = [pp.tile([128, 65], f16, name=f"vA_t{j}") for j in range(NJB)]
            vB_t = [pp.tile([128, 66], f16, name=f"vB_t{j}") for j in range(NJB)]
            attn_out = pp.tile([128, SEQ], f32)  # rows 0-63 A dims, 64-127 B dims
            den = pp.tile([66, SEQ], f32r)       # rows 64 (A), 65 (B)

            for j in range(NJB):
                nc.gpsimd.dma_start(out=vA_t[j][:, 64:65], in_=vconst_d[:, j, 0:1])
                nc.gpsimd.dma_start(out=vB_t[j][:, 64:66], in_=vconst_d[:, j, 1:3])
            nc.gpsimd.dma_start(out=den[64:66, SEQ // 2:], in_=ones2_d[:])
            nc.vector.memset(attn_out[64:128, SEQ // 2:], 0.0)

            # ---- phases 1+2 fused: projections interleaved into the
            # attention stream (shared scores PSUM tag, precise chunk deps) ----
            ps_s_cm = tc.tile_pool(name="ps_s", space="PSUM", bufs=2)
            ps_s = ps_s_cm.__enter__()
            p1_cm = tc.tile_pool(name="ph1", bufs=1)
            p1 = p1_cm.__enter__()
            pe_cm = tc.tile_pool(name="exps", bufs=6)
            pe = pe_cm.__enter__()
            ps_ov_cm = tc.tile_pool(name="ps_ov", bufs=1, space="PSUM")
            ps_ov = ps_ov_cm.__enter__()

            xt = p1.tile([128, FCH, SEQ], f16)
            xt_r = xt_d.rearrange("(c p) n -> p c n", p=128)
            for k in range(FCH):
                nc.sync.dma_start(out=xt[:, k, :], in_=xt_r[:, k, :])
            vT_t = [p1.tile([128, 512], f16, name=f"vT_t{i}") for i in range(SEQ // 512)]

            def proj_qk(nch):
                pq = ps_s.tile([128, W], f32, tag="s", name=f"pjqk{nch}")
                for k in range(FCH):
                    nc.tensor.matmul(pq[:, 0:512], wqk_sb[:, k, 0:128],
                                     xt[:, k, nch * 512:(nch + 1) * 512],
                                     start=(k == 0), stop=(k == FCH - 1))
                    nc.tensor.matmul(pq[:, 512:1024], wqk_sb[:, k, 128:256],
                                     xt[:, k, nch * 512:(nch + 1) * 512],
                                     start=(k == 0), stop=(k == FCH - 1))
                nc.vector.tensor_copy(q_t[nch][:], pq[:, 0:512])
                nc.vector.tensor_copy(k_t[nch][:], pq[:, 512:1024])
                nc.gpsimd.dma_start(out=q2_t[nch][64:128, :], in_=q_t[nch][0:64, :])
                nc.gpsimd.dma_start(out=k2_t[nch][64:128, :], in_=k_t[nch][0:64, :])

            def proj_v(nch):
                pv = ps_s.tile([128, W], f32, tag="s", name=f"pjv{nch}")
                for k in range(FCH):
                    nc.tensor.matmul(pv[:, 0:512], wv_sb[:, k, :],
                                     xt[:, k, nch * 512:(nch + 1) * 512],
                                     start=(k == 0), stop=(k == FCH - 1))
                nc.vector.tensor_copy(vT_t[nch][:], pv[:, 0:512])
                ptt = ps_s.tile([128, 8, 128], f16, tag="s", name=f"ptr{nch}")
                for q in range(4):
                    jb = nch * 4 + q
                    nc.tensor.transpose(ptt[:, q, :], vT_t[nch][:, q * 128:(q + 1) * 128], id_sb[:])
                    nc.vector.tensor_copy(vA_t[jb][:, 0:64], ptt[:, q, 0:64])
                    nc.vector.tensor_copy(vB_t[jb][:, 0:64], ptt[:, q, 64:128])

            pairs = [
                (("A", 0, q_t, k_t, vA_t), ("B", 0, q_t, k_t, vB_t)),
                (("A", 1, q_t, k_t, vA_t), ("B", 1, q_t, k_t, vB_t)),
                (("A", 2, q_t, k_t, vA_t), ("A2", 3, q2_t, k2_t, vA_t)),
            ]

            # all projections up front (they share the scores PSUM tag, so no
            # bank-reuse barrier separates them from the attention stream)
            for n in range(8):
                proj_qk(n)
            for n in range(8):
                proj_v(n)

            for pi, (c1, c2) in enumerate(pairs):
                ov1 = ps_ov.tile([65, W], f32, tag="ov1", name=f"ov1_{pi}")
                ov2 = ps_ov.tile([66, W], f32, tag="ov2", name=f"ov2_{pi}")
                ovs = (ov1, ov2)
                for jb in range(NJB):
                    jc, jo = jb // 4, (jb % 4) * 128
                    etiles = []
                    for h in range(NH):
                        sp = ps_s.tile([128, W], f32, tag="s", name=f"s{pi}_{jb}_{h}")
                        for ci, (_, ic, qt, kt, _) in enumerate((c1, c2)):
                            base = ci * 64
                            nc.tensor.matmul(
                                sp[:, ci * 512:(ci + 1) * 512],
                                kt[jc][base:base + 64, jo:jo + 128],
                                qt[ic * NH + h][base:base + 64, :],
                                start=True, stop=True)
                        e = pe.tile([128, W], f16, tag="e", name=f"e{pi}_{jb}_{h}")
                        nc.scalar.activation(out=e[:], in_=sp[:], func=EXP, scale=0.125)
                        etiles.append(e)
                    for h, e in enumerate(etiles):
                        for ci, (_, ic, _, _, vt) in enumerate((c1, c2)):
                            m = vt[jb].shape[1]
                            nc.tensor.matmul(ovs[ci][:m, h * 512:(h + 1) * 512],
                                             vt[jb][:], e[:, ci * 512:(ci + 1) * 512],
                                             start=(jb == 0), stop=(jb == NJB - 1))
                # drain accumulators to SBUF
                (n1, ic1, _, _, _), (n2, ic2, _, _, _) = c1, c2
                p10, p20 = ic1 * W, ic2 * W
                nc.vector.tensor_copy(attn_out[0:64, p10:p10 + W], ov1[0:64, :])
                if n2 == "B":
                    nc.vector.tensor_copy(den[64:66, p20:p20 + W], ov2[64:66, :])
                    ovb_sb = pe.tile([64, W], f32, tag="ovb_sb", name=f"ovb_sb{pi}", bufs=2)
                    nc.vector.tensor_copy(ovb_sb[:], ov2[0:64, :])
                    nc.gpsimd.dma_start(out=attn_out[64:128, p20:p20 + W], in_=ovb_sb[:])
                    nc.vector.tensor_copy(den[64:65, p10:p10 + W], ov1[64:65, :])
                else:
                    nc.vector.tensor_copy(attn_out[0:64, p20:p20 + W], ov2[0:64, :].bitcast(f32))
                    nc.vector.tensor_copy(den[64:65, p10:p10 + W], ov1[64:65, :])
                    nc.vector.tensor_copy(den[64:65, p20:p20 + W], ov2[64:65, :])

            ps_ov_cm.__exit__(None, None, None)
            pe_cm.__exit__(None, None, None)
            p1_cm.__exit__(None, None, None)
            ps_s_cm.__exit__(None, None, None)

            # ---- phase 3: normalize + output projection ----
            with tc.tile_pool(name="ph3", bufs=2) as p3, \
                 tc.tile_pool(name="ps_bc", bufs=4, space="PSUM") as ps_bc, \
                 tc.tile_pool(name="ps_o", bufs=4, space="PSUM") as ps_o:
                NT = SEQ // 512
                bcs, rcs = [], []
                for t in range(NT):
                    t0 = t * 512
                    bc = ps_bc.tile([128, 512], f32, tag="bc", name=f"bc{t}", bufs=4)
                    nc.tensor.matmul(bc[:], sel_sb[64:66, :], den[64:66, t0:t0 + 512],
                                     start=True, stop=True)
                    rc = p3.tile([128, 512], f32, tag="rc", name=f"rc{t}", bufs=4)
                    nc.vector.reciprocal_approx_fast(out=rc[:], in_=bc[:])
                    rcs.append(rc)
                for t in range(NT):
                    t0 = t * 512
                    nm = p3.tile([128, 512], f16, tag="nm", name=f"nm{t}", bufs=3)
                    nc.vector.tensor_tensor(out=nm[:], in0=attn_out[:, t0:t0 + 512],
                                            in1=rcs[t][:], op=mybir.AluOpType.mult)
                    for fb in range(FCH):
                        po = ps_o.tile([128, 512], f32, tag="po", name=f"po{t}_{fb}")
                        nc.tensor.matmul(po[:], wo_sb[:, fb * 128:(fb + 1) * 128], nm[:],
                                         start=True, stop=True)
                        ob = p3.tile([128, 512], f32, tag="ob", name=f"ob{t}_{fb}", bufs=6)
                        if fb % 2 == 0:
                            nc.vector.tensor_copy(ob[:], po[:])
                        else:
                            nc.scalar.copy(ob[:], po[:])
                        nc.sync.dma_start(out=out_d[fb * 128:(fb + 1) * 128, t0:t0 + 512],
                                          in_=ob[:])

    nc.compile()
    return nc


def _get_program():
    global _PROGRAM
    if _PROGRAM is None:
        _PROGRAM = _build_program()
    return _PROGRAM


def kernel(x: np.ndarray, w_qkv: np.ndarray, w_out: np.ndarray) -> np.ndarray:
    global LAST_RESULT
    import os
    try:
        import antenv.axon_hooks  # noqa: F401
    except ImportError:
        # without the NTFF hook, a leaked BASS_TRACE=1 would crash the
        # axon trace path inside run_bass_kernel_spmd
        os.environ["BASS_NEVER_TRACE"] = "1"
    from concourse.bass_utils import run_bass_kernel_spmd

    nc = _get_program()
    x2 = np.ascontiguousarray(x[0], dtype=np.float32)          # [SEQ, F]
    w_qkv = np.asarray(w_qkv, dtype=np.float32)                # [2304, F]
    w_out = np.asarray(w_out, dtype=np.float32)                # [F, 768]

    # per-head slices of w_qkv rows: o = h*192 + d*3 + {0:q, 1:k, 2:v}
    def wslice(h, which):
        return w_qkv[h * 192 + which:(h + 1) * 192:3, :]       # [64, F]

    sel = np.zeros((2, 128), dtype=np.float32)
    sel[0, 0:64] = 1.0
    sel[1, 64:128] = 1.0
    ident = np.eye(128, dtype=np.float16)
    vconst = np.zeros((128, NJB, 3), dtype=np.float16)
    vconst[:, :, 0] = 1.0
    vconst[:, :, 2] = 1.0
    ones2 = np.ones((2, SEQ // 2), dtype=np.float32)

    xt_plain = np.ascontiguousarray(x2.T.astype(np.float16))   # [F, SEQ]
    xt_rolled = np.ascontiguousarray(np.roll(x2, -SEQ // 2, axis=0).T.astype(np.float16))

    in_maps = []
    rolls = []
    for c in range(N_CORES):
        hA = c
        hB = 8 + c // 2
        roll = (SEQ // 2) if (c % 2) else 0
        rolls.append(roll)
        wqk = np.ascontiguousarray(np.concatenate(
            [wslice(hA, 0), wslice(hB, 0), wslice(hA, 1), wslice(hB, 1)],
            axis=0).T.astype(np.float16))
        wv = np.ascontiguousarray(np.concatenate(
            [wslice(hA, 2), wslice(hB, 2)], axis=0).T.astype(np.float16))
        cols = list(range(hA * 64, hA * 64 + 64)) + list(range(hB * 64, hB * 64 + 64))
        wo = np.ascontiguousarray(w_out[:, cols].T.astype(np.float16))  # [128, F]
        in_maps.append({
            "xt": xt_rolled if roll else xt_plain,
            "wqk": wqk, "wv": wv, "wo": wo, "sel": sel, "ident": ident,
            "vconst": vconst, "ones2": ones2,
        })

    res = run_bass_kernel_spmd(nc, in_maps, list(range(N_CORES)))
    LAST_RESULT = res

    acc = np.zeros((SEQ, N_FEATS), dtype=np.float64)
    for c in range(N_CORES):
        part = res.results[c]["out"]                           # [F, SEQ]
        if rolls[c]:
            part = np.roll(part, rolls[c], axis=1)
        acc += part.T.astype(np.float64)
    return acc.astype(np.float32)[None]
